# revision 3
# baseline (speedup 1.0000x reference)
"""Trainium2 Bass kernel for nn_KATLayer (KAT basis-function layer).

out[b,o] = sum_{i,n} exp(-z^2) * (1 + erf(alpha*z/sqrt(2))) * w[i,o,n]
  z = (x[b,i] - c[i,o,n]) / (|sigma|+1e-8),  c = |scale|*mx_start + mx_train

Sharding: output dim O split across 8 cores (OS=64 per core). Per core the
16.7M basis evaluations are tiled as partitions=i (KC=4 chunks of 128),
free=(o_local,n)=1024, one logical tile per (k,b); tiles run in groups of
B2=4 consecutive b sharing one k so per-k constants broadcast across the
group dim with stride-0 APs and fp16 DVE ops run 4096-wide at 2x rate.

Per group (host precomputes re32=rinv fp32, cre32=c*rinv fp32,
aa=alpha/sqrt2 fp16, wt=w*sqrt(pi)/2 fp16):
  zm = re32*x_b - cre32    (4x STT fp32->fp16; zm = +z, high precision)
  e  = DerivErf(zm)        (ACT)          um = zm*aa  (TT fp16 2x)
  t  = Erf(um)             (ACT)
  t1 = t+1   (TS fp16 imm,imm 4x, in-place on t)
  q  = e*t1  (TT 2x, out in t-tile)   p = q*wt (TT 2x in-place / GPSIMD)
  psum[b,(o,n)] += onehot_b.T @ p     (PE reduces i)
  out = reduce_n(psum)                (DVE, once)

Activations are phase-batched PG groups at a time (all DerivErf, then all
Erf) to amortize the ~1.3us ACT table switch; the DVE tail work (t1/q/p)
of superstep s is emitted after the zm-phase of superstep s+1 so the
in-order DVE queue never stalls waiting on ACT. p runs on GPSIMD (plain
fp16 TT) to off-load the DVE, which is the bottleneck engine.
"""
import sys

sys.path.insert(0, "/opt/trn_rl_repo")
import math

import numpy as np

B, I, O, N = 32, 512, 512, 16
NCORES = 8
OS = O // NCORES          # 64 output dims per core
KC = I // 128             # 4 i-chunks
P = 128
FREE = OS * N             # 1024
B2 = 4                    # b-tiles fused per instruction group
PG = 3                    # groups per activation phase batch
INV_SQRT2 = 0.7071067811865476
SQRT_PI_2 = math.sqrt(math.pi) / 2.0

P_GPSIMD = True           # p-multiply on GPSIMD instead of DVE
Q_GPSIMD_EVERY = 0        # every k-th group also does q on GPSIMD (0=off)

_CACHE = {}
LAST_RESULTS = None


def _build_nc():
    import concourse.bacc as bacc
    import concourse.mybir as mybir
    from concourse import tile

    fp32 = mybir.dt.float32
    fp16 = mybir.dt.float16
    AF = mybir.ActivationFunctionType
    ALU = mybir.AluOpType

    nc = bacc.Bacc(
        "TRN2", target_bir_lowering=False, debug=False, num_devices=NCORES
    )
    cre_d = nc.dram_tensor("cre", [KC, P, FREE], fp32, kind="ExternalInput")
    re_d = nc.dram_tensor("re", [KC, P, FREE], fp32, kind="ExternalInput")
    aa_d = nc.dram_tensor("aa", [KC, P, FREE], fp16, kind="ExternalInput")
    wt_d = nc.dram_tensor("wt", [KC, P, FREE], fp16, kind="ExternalInput")
    x_d = nc.dram_tensor("x", [KC, P, B], fp32, kind="ExternalInput")
    oh_d = nc.dram_tensor("oh", [P, B, B], fp16, kind="ExternalInput")
    out_d = nc.dram_tensor("out", [B, OS], fp32, kind="ExternalOutput")

    groups = [(k, B2 * bg) for k in range(KC) for bg in range(B // B2)]
    n_tiles = KC * B

    with tile.TileContext(nc) as tc:
        with (
            tc.tile_pool(name="const", bufs=1) as cp,
            tc.tile_pool(name="zmp", bufs=PG) as zmp,
            tc.tile_pool(name="ump", bufs=PG) as ump,
            tc.tile_pool(name="ep", bufs=2 * PG) as ep,
            tc.tile_pool(name="tp", bufs=2 * PG) as tp,
            tc.tile_pool(name="psum", bufs=1, space="PSUM") as psp,
            tc.tile_pool(name="outp", bufs=1) as op_,
        ):
            cre_sb, re_sb, aa_sb, wt_sb = [], [], [], []
            for k in range(KC):
                for lst, dram, nm, dt_ in (
                    (cre_sb, cre_d, "cre", fp32),
                    (re_sb, re_d, "re", fp32),
                    (aa_sb, aa_d, "aa", fp16),
                    (wt_sb, wt_d, "wt", fp16),
                ):
                    t = cp.tile([P, FREE], dt_, tag=f"{nm}{k}")
                    nc.sync.dma_start(t[:], dram[k])
                    lst.append(t)
            x_sb = cp.tile([P, KC * B], fp32, tag="x")
            for k in range(KC):
                nc.sync.dma_start(x_sb[:, k * B : (k + 1) * B], x_d[k])
            oh_sb = cp.tile([P, B, B], fp16, tag="oh")
            nc.sync.dma_start(oh_sb[:], oh_d[:])

            psum_t = psp.tile([B, OS, N], fp32)
            out_sb = op_.tile([B, OS], fp32)

            state = {"n_mm": 0}

            def zm_phase(g):
                k, b0 = groups[g]
                zm = zmp.tile([P, B2, FREE], fp16, tag="zm")
                for j in range(B2):
                    xcol = x_sb[:, k * B + b0 + j : k * B + b0 + j + 1]
                    nc.vector.scalar_tensor_tensor(
                        zm[:, j, :], re_sb[k][:], xcol, cre_sb[k][:],
                        op0=ALU.mult, op1=ALU.subtract,
                    )
                return zm

            def tail(g, e, t_tile):
                k, b0 = groups[g]
                wt_b = wt_sb[k][:].unsqueeze(1).broadcast_to([P, B2, FREE])
                nc.vector.tensor_scalar(
                    t_tile[:], t_tile[:], 1.0, 1.0, op0=ALU.add, op1=ALU.mult
                )
                q_eng = (nc.gpsimd if Q_GPSIMD_EVERY
                         and g % Q_GPSIMD_EVERY == Q_GPSIMD_EVERY - 1
                         else nc.vector)
                q_eng.tensor_tensor(t_tile[:], e[:], t_tile[:], op=ALU.mult)
                p_eng = nc.gpsimd if P_GPSIMD else nc.vector
                p_eng.tensor_tensor(t_tile[:], t_tile[:], wt_b, op=ALU.mult)
                for j in range(B2):
                    b = b0 + j
                    for h in range(2):
                        nc.tensor.matmul(
                            psum_t[:, 32 * h : 32 * (h + 1), :],
                            oh_sb[:, b, :],
                            t_tile[:, j, 512 * h : 512 * (h + 1)],
                            start=(state["n_mm"] < 2),
                            stop=(state["n_mm"] >= 2 * n_tiles - 2),
                        )
                        state["n_mm"] += 1

            pending = []
            for s0 in range(0, len(groups), PG):
                ss = list(range(s0, min(s0 + PG, len(groups))))
                zms = [zm_phase(g) for g in ss]
                # previous superstep's DVE tail work goes here: by now ACT
                # has drained those groups while DVE was busy with zm above
                for args in pending:
                    tail(*args)
                pending = []
                es = []
                for g, zm in zip(ss, zms):
                    e = ep.tile([P, B2, FREE], fp16, tag="e")
                    nc.scalar.activation(e[:], zm[:], AF.Derivative_Erf)
                    es.append(e)
                ums = []
                for g, zm in zip(ss, zms):
                    k, _ = groups[g]
                    aa_b = aa_sb[k][:].unsqueeze(1).broadcast_to([P, B2, FREE])
                    um = ump.tile([P, B2, FREE], fp16, tag="um")
                    nc.vector.tensor_tensor(um[:], zm[:], aa_b, op=ALU.mult)
                    ums.append(um)
                for g, e, um in zip(ss, es, ums):
                    t_tile = tp.tile([P, B2, FREE], fp16, tag="t")
                    nc.scalar.activation(t_tile[:], um[:], AF.Erf)
                    pending.append((g, e, t_tile))
            for args in pending:
                tail(*args)

            nc.vector.tensor_reduce(
                out_sb[:], psum_t[:], axis=mybir.AxisListType.X, op=ALU.add
            )
            nc.sync.dma_start(out_d[:], out_sb[:])

    nc.compile()
    return nc


def _prep_inputs(x, mx_train, scale, sigma, alpha, w, mx_start):
    c = (np.abs(scale)[:, :, None] * mx_start[None, None, :]
         + mx_train[:, :, None]).astype(np.float32)
    rinv = (1.0 / (np.abs(sigma) + 1e-8)).astype(np.float32)
    cre = (c * rinv).astype(np.float32)
    aa = (alpha * INV_SQRT2).astype(np.float16)
    wt = (w * SQRT_PI_2).astype(np.float16)
    xT = np.ascontiguousarray(x.T.reshape(KC, P, B)).astype(np.float32)
    oh = np.ascontiguousarray(
        np.broadcast_to(np.eye(B, dtype=np.float16), (P, B, B)))

    in_maps = []
    for d in range(NCORES):
        sl = slice(d * OS, (d + 1) * OS)
        in_maps.append({
            "cre": np.ascontiguousarray(cre[:, sl].reshape(KC, P, FREE)),
            "re": np.ascontiguousarray(rinv[:, sl].reshape(KC, P, FREE)),
            "aa": np.ascontiguousarray(aa[:, sl].reshape(KC, P, FREE)),
            "wt": np.ascontiguousarray(wt[:, sl].reshape(KC, P, FREE)),
            "x": xT,
            "oh": oh,
        })
    return in_maps


def kernel(x, mx_train, scale, sigma, alpha, w, mx_start, _trace=False):
    global LAST_RESULTS
    from concourse.bass_utils import run_bass_kernel_spmd

    if "nc" not in _CACHE:
        _CACHE["nc"] = _build_nc()
    nc = _CACHE["nc"]
    in_maps = _prep_inputs(
        np.asarray(x, np.float32), np.asarray(mx_train, np.float32),
        np.asarray(scale, np.float32), np.asarray(sigma, np.float32),
        np.asarray(alpha, np.float32), np.asarray(w, np.float32),
        np.asarray(mx_start, np.float32),
    )
    res = run_bass_kernel_spmd(nc, in_maps, core_ids=list(range(NCORES)),
                               trace=_trace)
    LAST_RESULTS = res
    return np.concatenate([r["out"] for r in res.results], axis=1)


# revision 4
# speedup vs baseline: 1.4392x; 1.4392x over previous
"""Trainium2 Bass kernel for nn_KATLayer (KAT basis-function layer).

out[b,o] = sum_{i,n} exp(-z^2) * (1 + erf(alpha*z/sqrt(2))) * w[i,o,n]
  z = (x[b,i] - c[i,o,n]) / (|sigma|+1e-8),  c = |scale|*mx_start + mx_train

Sharding: output dim O split across 8 cores (OS=64 per core). Per core the
16.7M basis evaluations are tiled as partitions=i (KC=4 chunks of 128),
free=(o_local,n)=1024, one logical tile per (k,b); tiles run in groups of
B2=4 consecutive b sharing one k so per-k constants broadcast across the
group dim with stride-0 APs and fp16 DVE ops run 4096-wide at 2x rate.

Per group (host precomputes re32=rinv fp32, cre32=c*rinv fp32,
aa=alpha/sqrt2 fp16, wt=w*sqrt(pi)/2 fp16):
  zm = re32*x_b - cre32    (4x STT fp32->fp16; zm = +z, high precision)
  e  = DerivErf(zm)        (ACT)          um = zm*aa  (TT fp16 2x)
  t  = Erf(um)             (ACT)
  t1 = t+1   (TS fp16 imm,imm 4x, in-place on t)
  q  = e*t1  (TT 2x, out in t-tile)   p = q*wt (TT 2x in-place / GPSIMD)
  psum[b,(o,n)] += onehot_b.T @ p     (PE reduces i)
  out = reduce_n(psum)                (DVE, once)

Activations are phase-batched PG groups at a time (all DerivErf, then all
Erf) to amortize the ~1.3us ACT table switch; the DVE tail work (t1/q/p)
of superstep s is emitted after the zm-phase of superstep s+1 so the
in-order DVE queue never stalls waiting on ACT. p runs on GPSIMD (plain
fp16 TT) to off-load the DVE, which is the bottleneck engine.
"""
import sys

sys.path.insert(0, "/opt/trn_rl_repo")
import math

import numpy as np

B, I, O, N = 32, 512, 512, 16
NCORES = 8
OS = O // NCORES          # 64 output dims per core
KC = I // 128             # 4 i-chunks
P = 128
FREE = OS * N             # 1024
B2 = 4                    # b-tiles fused per instruction group
PG = 3                    # groups per activation phase batch
INV_SQRT2 = 0.7071067811865476
SQRT_PI_2 = math.sqrt(math.pi) / 2.0

T1_ON_ACT = True          # t+1 via ACT Identity(bias=1) instead of DVE TS

_CACHE = {}
LAST_RESULTS = None


def _build_nc():
    import concourse.bacc as bacc
    import concourse.mybir as mybir
    from concourse import tile

    fp32 = mybir.dt.float32
    fp16 = mybir.dt.float16
    AF = mybir.ActivationFunctionType
    ALU = mybir.AluOpType

    nc = bacc.Bacc(
        "TRN2", target_bir_lowering=False, debug=False, num_devices=NCORES
    )
    cre_d = nc.dram_tensor("cre", [KC, P, FREE], fp32, kind="ExternalInput")
    re_d = nc.dram_tensor("re", [KC, P, FREE], fp32, kind="ExternalInput")
    aa_d = nc.dram_tensor("aa", [KC, P, FREE], fp16, kind="ExternalInput")
    wt_d = nc.dram_tensor("wt", [KC, P, FREE], fp16, kind="ExternalInput")
    x_d = nc.dram_tensor("x", [KC, P, B], fp32, kind="ExternalInput")
    oh_d = nc.dram_tensor("oh", [P, B, B], fp16, kind="ExternalInput")
    out_d = nc.dram_tensor("out", [B, OS], fp32, kind="ExternalOutput")

    groups = [(k, B2 * bg) for k in range(KC) for bg in range(B // B2)]
    n_tiles = KC * B

    with tile.TileContext(nc) as tc:
        with (
            tc.tile_pool(name="const", bufs=1) as cp,
            tc.tile_pool(name="zmp", bufs=PG) as zmp,
            tc.tile_pool(name="ump", bufs=PG) as ump,
            tc.tile_pool(name="ep", bufs=2 * PG) as ep,
            tc.tile_pool(name="tp", bufs=2 * PG) as tp,
            tc.tile_pool(name="psum", bufs=1, space="PSUM") as psp,
            tc.tile_pool(name="outp", bufs=1) as op_,
        ):
            cre_sb, re_sb, aa_sb, wt_sb = [], [], [], []
            for k in range(KC):
                for lst, dram, nm, dt_ in (
                    (cre_sb, cre_d, "cre", fp32),
                    (re_sb, re_d, "re", fp32),
                    (aa_sb, aa_d, "aa", fp16),
                    (wt_sb, wt_d, "wt", fp16),
                ):
                    t = cp.tile([P, FREE], dt_, tag=f"{nm}{k}")
                    nc.sync.dma_start(t[:], dram[k])
                    lst.append(t)
            x_sb = cp.tile([P, KC * B], fp32, tag="x")
            for k in range(KC):
                nc.sync.dma_start(x_sb[:, k * B : (k + 1) * B], x_d[k])
            oh_sb = cp.tile([P, B, B], fp16, tag="oh")
            nc.sync.dma_start(oh_sb[:], oh_d[:])

            psum_t = psp.tile([B, OS, N], fp32)
            out_sb = op_.tile([B, OS], fp32)

            state = {"n_mm": 0}

            def zm_phase(g):
                k, b0 = groups[g]
                zm = zmp.tile([P, B2, FREE], fp16, tag="zm")
                for j in range(B2):
                    xcol = x_sb[:, k * B + b0 + j : k * B + b0 + j + 1]
                    nc.vector.scalar_tensor_tensor(
                        zm[:, j, :], re_sb[k][:], xcol, cre_sb[k][:],
                        op0=ALU.mult, op1=ALU.subtract,
                    )
                return zm

            def tail(g, e, t_tile):
                k, b0 = groups[g]
                wt_b = wt_sb[k][:].unsqueeze(1).broadcast_to([P, B2, FREE])
                if not T1_ON_ACT:
                    nc.vector.tensor_scalar(
                        t_tile[:], t_tile[:], 1.0, 1.0,
                        op0=ALU.add, op1=ALU.mult,
                    )
                nc.vector.tensor_tensor(t_tile[:], e[:], t_tile[:], op=ALU.mult)
                nc.vector.tensor_tensor(t_tile[:], t_tile[:], wt_b, op=ALU.mult)
                for j in range(B2):
                    b = b0 + j
                    for h in range(2):
                        nc.tensor.matmul(
                            psum_t[:, 32 * h : 32 * (h + 1), :],
                            oh_sb[:, b, :],
                            t_tile[:, j, 512 * h : 512 * (h + 1)],
                            start=(state["n_mm"] < 2),
                            stop=(state["n_mm"] >= 2 * n_tiles - 2),
                        )
                        state["n_mm"] += 1

            pending = []
            for s0 in range(0, len(groups), PG):
                ss = list(range(s0, min(s0 + PG, len(groups))))
                zms = [zm_phase(g) for g in ss]
                es = []
                for g, zm in zip(ss, zms):
                    e = ep.tile([P, B2, FREE], fp16, tag="e")
                    nc.scalar.activation(e[:], zm[:], AF.Derivative_Erf)
                    es.append(e)
                ums = []
                for g, zm in zip(ss, zms):
                    k, _ = groups[g]
                    aa_b = aa_sb[k][:].unsqueeze(1).broadcast_to([P, B2, FREE])
                    um = ump.tile([P, B2, FREE], fp16, tag="um")
                    nc.vector.tensor_tensor(um[:], zm[:], aa_b, op=ALU.mult)
                    ums.append(um)
                # previous superstep's DVE tail work goes here: by now ACT
                # has drained those groups while DVE was busy with zm above
                for args in pending:
                    tail(*args)
                pending = []
                for g, e, um in zip(ss, es, ums):
                    t_tile = tp.tile([P, B2, FREE], fp16, tag="t")
                    nc.scalar.activation(t_tile[:], um[:], AF.Erf)
                    if T1_ON_ACT:
                        nc.scalar.activation(
                            t_tile[:], t_tile[:], AF.Identity, bias=1.0
                        )
                    pending.append((g, e, t_tile))
            for args in pending:
                tail(*args)

            nc.vector.tensor_reduce(
                out_sb[:], psum_t[:], axis=mybir.AxisListType.X, op=ALU.add
            )
            nc.sync.dma_start(out_d[:], out_sb[:])

    nc.compile()
    return nc


def _prep_inputs(x, mx_train, scale, sigma, alpha, w, mx_start):
    c = (np.abs(scale)[:, :, None] * mx_start[None, None, :]
         + mx_train[:, :, None]).astype(np.float32)
    rinv = (1.0 / (np.abs(sigma) + 1e-8)).astype(np.float32)
    cre = (c * rinv).astype(np.float32)
    aa = (alpha * INV_SQRT2).astype(np.float16)
    wt = (w * SQRT_PI_2).astype(np.float16)
    xT = np.ascontiguousarray(x.T.reshape(KC, P, B)).astype(np.float32)
    oh = np.ascontiguousarray(
        np.broadcast_to(np.eye(B, dtype=np.float16), (P, B, B)))

    in_maps = []
    for d in range(NCORES):
        sl = slice(d * OS, (d + 1) * OS)
        in_maps.append({
            "cre": np.ascontiguousarray(cre[:, sl].reshape(KC, P, FREE)),
            "re": np.ascontiguousarray(rinv[:, sl].reshape(KC, P, FREE)),
            "aa": np.ascontiguousarray(aa[:, sl].reshape(KC, P, FREE)),
            "wt": np.ascontiguousarray(wt[:, sl].reshape(KC, P, FREE)),
            "x": xT,
            "oh": oh,
        })
    return in_maps


def kernel(x, mx_train, scale, sigma, alpha, w, mx_start, _trace=False):
    global LAST_RESULTS
    from concourse.bass_utils import run_bass_kernel_spmd

    if "nc" not in _CACHE:
        _CACHE["nc"] = _build_nc()
    nc = _CACHE["nc"]
    in_maps = _prep_inputs(
        np.asarray(x, np.float32), np.asarray(mx_train, np.float32),
        np.asarray(scale, np.float32), np.asarray(sigma, np.float32),
        np.asarray(alpha, np.float32), np.asarray(w, np.float32),
        np.asarray(mx_start, np.float32),
    )
    res = run_bass_kernel_spmd(nc, in_maps, core_ids=list(range(NCORES)),
                               trace=_trace)
    LAST_RESULTS = res
    return np.concatenate([r["out"] for r in res.results], axis=1)


# revision 6
# speedup vs baseline: 1.5249x; 1.0595x over previous
"""Trainium2 Bass kernel for nn_KATLayer (KAT basis-function layer).

out[b,o] = sum_{i,n} exp(-z^2) * (1 + erf(alpha*z/sqrt(2))) * w[i,o,n]
  z = (x[b,i] - c[i,o,n]) / (|sigma|+1e-8),  c = |scale|*mx_start + mx_train

Sharding: output dim O split across 8 cores (OS=64 per core). Per core the
16.7M basis evaluations are tiled as partitions=i (KC=4 chunks of 128),
free=(o_local,n)=1024, one logical tile per (k,b); tiles run in groups of
B2=4 consecutive b sharing one k so per-k constants broadcast across the
group dim with stride-0 APs and fp16 DVE ops run 4096-wide at 2x rate.

Per group (host precomputes re32=rinv fp32, cre32=c*rinv fp32,
aa=alpha/sqrt2 fp16, wt=w*sqrt(pi)/2 fp16):
  zm = re32*x_b - cre32    (4x STT fp32->fp16; zm = +z, high precision)
  e  = DerivErf(zm)        (ACT)          um = zm*aa  (TT fp16 2x)
  t  = Erf(um)             (ACT)
  t1 = t+1   (TS fp16 imm,imm 4x, in-place on t)
  q  = e*t1  (TT 2x, out in t-tile)   p = q*wt (TT 2x in-place / GPSIMD)
  psum[b,(o,n)] += onehot_b.T @ p     (PE reduces i)
  out = reduce_n(psum)                (DVE, once)

Activations are phase-batched PG groups at a time (all DerivErf, then all
Erf) to amortize the ~1.3us ACT table switch; the DVE tail work (t1/q/p)
of superstep s is emitted after the zm-phase of superstep s+1 so the
in-order DVE queue never stalls waiting on ACT. p runs on GPSIMD (plain
fp16 TT) to off-load the DVE, which is the bottleneck engine.
"""
import sys

sys.path.insert(0, "/opt/trn_rl_repo")
import math

import numpy as np

B, I, O, N = 32, 512, 512, 16
NCORES = 8
OS = O // NCORES          # 64 output dims per core
KC = I // 128             # 4 i-chunks
P = 128
FREE = OS * N             # 1024
B2 = 4                    # b-tiles fused per instruction group
PG = 3                    # groups per activation phase batch
INV_SQRT2 = 0.7071067811865476
SQRT_PI_2 = math.sqrt(math.pi) / 2.0

TWO_STREAM = True         # accumulate S(e*w)*t + S(e*w) via 2x matmuls

_CACHE = {}
LAST_RESULTS = None


def _build_nc():
    import concourse.bacc as bacc
    import concourse.mybir as mybir
    from concourse import tile

    fp32 = mybir.dt.float32
    fp16 = mybir.dt.float16
    AF = mybir.ActivationFunctionType
    ALU = mybir.AluOpType

    nc = bacc.Bacc(
        "TRN2", target_bir_lowering=False, debug=False, num_devices=NCORES
    )
    cre_d = nc.dram_tensor("cre", [KC, P, FREE], fp32, kind="ExternalInput")
    re_d = nc.dram_tensor("re", [KC, P, FREE], fp32, kind="ExternalInput")
    aa_d = nc.dram_tensor("aa", [KC, P, FREE], fp16, kind="ExternalInput")
    wt_d = nc.dram_tensor("wt", [KC, P, FREE], fp16, kind="ExternalInput")
    x_d = nc.dram_tensor("x", [KC, P, B], fp32, kind="ExternalInput")
    oh_d = nc.dram_tensor("oh", [P, B, B], fp16, kind="ExternalInput")
    out_d = nc.dram_tensor("out", [B, OS], fp32, kind="ExternalOutput")

    groups = [(k, B2 * bg) for k in range(KC) for bg in range(B // B2)]
    n_tiles = KC * B

    with tile.TileContext(nc) as tc:
        with (
            tc.tile_pool(name="const", bufs=1) as cp,
            tc.tile_pool(name="zmp", bufs=PG) as zmp,
            tc.tile_pool(name="ump", bufs=PG) as ump,
            tc.tile_pool(name="ep", bufs=2 * PG) as ep,
            tc.tile_pool(name="tp", bufs=2 * PG) as tp,
            tc.tile_pool(name="psum", bufs=1, space="PSUM") as psp,
            tc.tile_pool(name="outp", bufs=1) as op_,
        ):
            cre_sb, re_sb, aa_sb, wt_sb = [], [], [], []
            for k in range(KC):
                for lst, dram, nm, dt_ in (
                    (cre_sb, cre_d, "cre", fp32),
                    (re_sb, re_d, "re", fp32),
                    (aa_sb, aa_d, "aa", fp16),
                    (wt_sb, wt_d, "wt", fp16),
                ):
                    t = cp.tile([P, FREE], dt_, tag=f"{nm}{k}")
                    nc.sync.dma_start(t[:], dram[k])
                    lst.append(t)
            x_sb = cp.tile([P, KC * B], fp32, tag="x")
            for k in range(KC):
                nc.sync.dma_start(x_sb[:, k * B : (k + 1) * B], x_d[k])
            oh_sb = cp.tile([P, B, B], fp16, tag="oh")
            nc.sync.dma_start(oh_sb[:], oh_d[:])

            psum_t = psp.tile([B, OS, N], fp32)
            out_sb = op_.tile([B, OS], fp32)

            state = {"n_mm": 0}

            def zm_phase(g):
                k, b0 = groups[g]
                zm = zmp.tile([P, B2, FREE], fp16, tag="zm")
                for j in range(B2):
                    xcol = x_sb[:, k * B + b0 + j : k * B + b0 + j + 1]
                    nc.vector.scalar_tensor_tensor(
                        zm[:, j, :], re_sb[k][:], xcol, cre_sb[k][:],
                        op0=ALU.mult, op1=ALU.subtract,
                    )
                return zm

            n_mm_total = (4 if TWO_STREAM else 2) * n_tiles

            def mm(b, mov):
                for h in range(2):
                    nc.tensor.matmul(
                        psum_t[:, 32 * h : 32 * (h + 1), :],
                        oh_sb[:, b, :],
                        mov[:, 512 * h : 512 * (h + 1)],
                        start=(state["n_mm"] < 2),
                        stop=(state["n_mm"] >= n_mm_total - 2),
                    )
                    state["n_mm"] += 1

            def tail(g, e, t_tile):
                k, b0 = groups[g]
                wt_b = wt_sb[k][:].unsqueeze(1).broadcast_to([P, B2, FREE])
                if TWO_STREAM:
                    # ew = e*wt (in e-tile); s2 = ew*t (in t-tile)
                    nc.vector.tensor_tensor(e[:], e[:], wt_b, op=ALU.mult)
                    nc.vector.tensor_tensor(
                        t_tile[:], e[:], t_tile[:], op=ALU.mult)
                    for j in range(B2):
                        mm(b0 + j, t_tile[:, j, :])
                        mm(b0 + j, e[:, j, :])
                else:
                    nc.vector.tensor_tensor(
                        t_tile[:], e[:], t_tile[:], op=ALU.mult)
                    nc.vector.tensor_tensor(
                        t_tile[:], t_tile[:], wt_b, op=ALU.mult)
                    for j in range(B2):
                        mm(b0 + j, t_tile[:, j, :])

            pending = []
            for s0 in range(0, len(groups), PG):
                ss = list(range(s0, min(s0 + PG, len(groups))))
                zms = [zm_phase(g) for g in ss]
                es = []
                for g, zm in zip(ss, zms):
                    e = ep.tile([P, B2, FREE], fp16, tag="e")
                    nc.scalar.activation(e[:], zm[:], AF.Derivative_Erf)
                    es.append(e)
                ums = []
                for g, zm in zip(ss, zms):
                    k, _ = groups[g]
                    aa_b = aa_sb[k][:].unsqueeze(1).broadcast_to([P, B2, FREE])
                    um = ump.tile([P, B2, FREE], fp16, tag="um")
                    nc.vector.tensor_tensor(um[:], zm[:], aa_b, op=ALU.mult)
                    ums.append(um)
                # previous superstep's DVE tail work goes here: by now ACT
                # has drained those groups while DVE was busy with zm above
                for args in pending:
                    tail(*args)
                pending = []
                for g, e, um in zip(ss, es, ums):
                    t_tile = tp.tile([P, B2, FREE], fp16, tag="t")
                    nc.scalar.activation(t_tile[:], um[:], AF.Erf)
                    if not TWO_STREAM:
                        nc.scalar.activation(
                            t_tile[:], t_tile[:], AF.Identity, bias=1.0
                        )
                    pending.append((g, e, t_tile))
            for args in pending:
                tail(*args)

            nc.vector.tensor_reduce(
                out_sb[:], psum_t[:], axis=mybir.AxisListType.X, op=ALU.add
            )
            nc.sync.dma_start(out_d[:], out_sb[:])

    nc.compile()
    return nc


def _prep_inputs(x, mx_train, scale, sigma, alpha, w, mx_start):
    c = (np.abs(scale)[:, :, None] * mx_start[None, None, :]
         + mx_train[:, :, None]).astype(np.float32)
    rinv = (1.0 / (np.abs(sigma) + 1e-8)).astype(np.float32)
    cre = (c * rinv).astype(np.float32)
    aa = (alpha * INV_SQRT2).astype(np.float16)
    wt = (w * SQRT_PI_2).astype(np.float16)
    xT = np.ascontiguousarray(x.T.reshape(KC, P, B)).astype(np.float32)
    oh = np.ascontiguousarray(
        np.broadcast_to(np.eye(B, dtype=np.float16), (P, B, B)))

    in_maps = []
    for d in range(NCORES):
        sl = slice(d * OS, (d + 1) * OS)
        in_maps.append({
            "cre": np.ascontiguousarray(cre[:, sl].reshape(KC, P, FREE)),
            "re": np.ascontiguousarray(rinv[:, sl].reshape(KC, P, FREE)),
            "aa": np.ascontiguousarray(aa[:, sl].reshape(KC, P, FREE)),
            "wt": np.ascontiguousarray(wt[:, sl].reshape(KC, P, FREE)),
            "x": xT,
            "oh": oh,
        })
    return in_maps


def kernel(x, mx_train, scale, sigma, alpha, w, mx_start, _trace=False):
    global LAST_RESULTS
    from concourse.bass_utils import run_bass_kernel_spmd

    if "nc" not in _CACHE:
        _CACHE["nc"] = _build_nc()
    nc = _CACHE["nc"]
    in_maps = _prep_inputs(
        np.asarray(x, np.float32), np.asarray(mx_train, np.float32),
        np.asarray(scale, np.float32), np.asarray(sigma, np.float32),
        np.asarray(alpha, np.float32), np.asarray(w, np.float32),
        np.asarray(mx_start, np.float32),
    )
    res = run_bass_kernel_spmd(nc, in_maps, core_ids=list(range(NCORES)),
                               trace=_trace)
    LAST_RESULTS = res
    return np.concatenate([r["out"] for r in res.results], axis=1)


# revision 7
# speedup vs baseline: 1.5712x; 1.0304x over previous
"""Trainium2 Bass kernel for nn_KATLayer (KAT basis-function layer).

out[b,o] = sum_{i,n} exp(-z^2) * (1 + erf(alpha*z/sqrt(2))) * w[i,o,n]
  z = (x[b,i] - c[i,o,n]) / (|sigma|+1e-8),  c = |scale|*mx_start + mx_train

Sharding: output dim O split across 8 cores (OS=64 per core). Per core the
16.7M basis evaluations are tiled as partitions=i (KC=4 chunks of 128),
free=(o_local,n)=1024, one logical tile per (k,b); tiles run in groups of
B2=4 consecutive b sharing one k so per-k constants broadcast across the
group dim with stride-0 APs and fp16 DVE ops run 4096-wide at 2x rate.

Per group (host precomputes re32=rinv fp32, cre32=c*rinv fp32,
aa=alpha/sqrt2 fp16, wt=w*sqrt(pi)/2 fp16):
  zm = re32*x_b - cre32    (4x STT fp32->fp16; zm = +z, high precision)
  e  = DerivErf(zm)        (ACT)          um = zm*aa  (TT fp16 2x)
  t  = Erf(um)             (ACT)
  t1 = t+1   (TS fp16 imm,imm 4x, in-place on t)
  q  = e*t1  (TT 2x, out in t-tile)   p = q*wt (TT 2x in-place / GPSIMD)
  psum[b,(o,n)] += onehot_b.T @ p     (PE reduces i)
  out = reduce_n(psum)                (DVE, once)

Activations are phase-batched PG groups at a time (all DerivErf, then all
Erf) to amortize the ~1.3us ACT table switch; the DVE tail work (t1/q/p)
of superstep s is emitted after the zm-phase of superstep s+1 so the
in-order DVE queue never stalls waiting on ACT. p runs on GPSIMD (plain
fp16 TT) to off-load the DVE, which is the bottleneck engine.
"""
import sys

sys.path.insert(0, "/opt/trn_rl_repo")
import math

import numpy as np

B, I, O, N = 32, 512, 512, 16
NCORES = 8
OS = O // NCORES          # 64 output dims per core
KC = I // 128             # 4 i-chunks
P = 128
FREE = OS * N             # 1024
B2 = 4                    # b-tiles fused per instruction group
PG = 3                    # groups per activation phase batch
INV_SQRT2 = 0.7071067811865476
SQRT_PI_2 = math.sqrt(math.pi) / 2.0

TWO_STREAM = True         # accumulate S(e*w)*t + S(e*w) via 2x matmuls

_CACHE = {}
LAST_RESULTS = None


def _build_nc():
    import concourse.bacc as bacc
    import concourse.mybir as mybir
    from concourse import tile

    fp32 = mybir.dt.float32
    fp16 = mybir.dt.float16
    AF = mybir.ActivationFunctionType
    ALU = mybir.AluOpType

    nc = bacc.Bacc(
        "TRN2", target_bir_lowering=False, debug=False, num_devices=NCORES
    )
    cre_d = nc.dram_tensor("cre", [KC, P, FREE], fp32, kind="ExternalInput")
    re_d = nc.dram_tensor("re", [KC, P, FREE], fp32, kind="ExternalInput")
    aa_d = nc.dram_tensor("aa", [KC, P, FREE], fp16, kind="ExternalInput")
    wt_d = nc.dram_tensor("wt", [KC, P, FREE], fp16, kind="ExternalInput")
    x_d = nc.dram_tensor("x", [KC, P, B], fp32, kind="ExternalInput")
    oh_d = nc.dram_tensor("oh", [P, B, B], fp16, kind="ExternalInput")
    out_d = nc.dram_tensor("out", [B, OS], fp32, kind="ExternalOutput")

    groups = [(k, B2 * bg) for k in range(KC) for bg in range(B // B2)]
    n_tiles = KC * B

    with tile.TileContext(nc) as tc:
        with (
            tc.tile_pool(name="const", bufs=1) as cp,
            tc.tile_pool(name="zmp", bufs=PG) as zmp,
            tc.tile_pool(name="ump", bufs=PG) as ump,
            tc.tile_pool(name="ep", bufs=2 * PG) as ep,
            tc.tile_pool(name="tp", bufs=2 * PG) as tp,
            tc.tile_pool(name="psum", bufs=1, space="PSUM") as psp,
            tc.tile_pool(name="outp", bufs=1) as op_,
        ):
            # x + oh first (first STT needs x), then per-k in consumption
            # order so the first superstep's inputs land earliest
            x_sb = cp.tile([P, KC * B], fp32, tag="x")
            for k in range(KC):
                nc.sync.dma_start(x_sb[:, k * B : (k + 1) * B], x_d[k])
            oh_sb = cp.tile([P, B, B], fp16, tag="oh")
            nc.sync.dma_start(oh_sb[:], oh_d[:])
            cre_sb, re_sb, aa_sb, wt_sb = [], [], [], []
            for k in range(KC):
                for lst, dram, nm, dt_ in (
                    (cre_sb, cre_d, "cre", fp32),
                    (re_sb, re_d, "re", fp32),
                    (aa_sb, aa_d, "aa", fp16),
                    (wt_sb, wt_d, "wt", fp16),
                ):
                    t = cp.tile([P, FREE], dt_, tag=f"{nm}{k}")
                    nc.sync.dma_start(t[:], dram[k])
                    lst.append(t)

            psum_t = psp.tile([B, OS, N], fp32)
            out_sb = op_.tile([B, OS], fp32)

            state = {"n_mm": 0}

            def zm_phase(g):
                k, b0 = groups[g]
                zm = zmp.tile([P, B2, FREE], fp16, tag="zm")
                for j in range(B2):
                    xcol = x_sb[:, k * B + b0 + j : k * B + b0 + j + 1]
                    nc.vector.scalar_tensor_tensor(
                        zm[:, j, :], re_sb[k][:], xcol, cre_sb[k][:],
                        op0=ALU.mult, op1=ALU.subtract,
                    )
                return zm

            n_mm_total = (4 if TWO_STREAM else 2) * n_tiles

            def mm(b, mov):
                for h in range(2):
                    nc.tensor.matmul(
                        psum_t[:, 32 * h : 32 * (h + 1), :],
                        oh_sb[:, b, :],
                        mov[:, 512 * h : 512 * (h + 1)],
                        start=(state["n_mm"] < 2),
                        stop=(state["n_mm"] >= n_mm_total - 2),
                    )
                    state["n_mm"] += 1

            def tail(g, e, t_tile):
                k, b0 = groups[g]
                wt_b = wt_sb[k][:].unsqueeze(1).broadcast_to([P, B2, FREE])
                if TWO_STREAM:
                    # ew = e*wt (in e-tile); s2 = ew*t (in t-tile)
                    nc.vector.tensor_tensor(e[:], e[:], wt_b, op=ALU.mult)
                    nc.vector.tensor_tensor(
                        t_tile[:], e[:], t_tile[:], op=ALU.mult)
                    for j in range(B2):
                        mm(b0 + j, t_tile[:, j, :])
                        mm(b0 + j, e[:, j, :])
                else:
                    nc.vector.tensor_tensor(
                        t_tile[:], e[:], t_tile[:], op=ALU.mult)
                    nc.vector.tensor_tensor(
                        t_tile[:], t_tile[:], wt_b, op=ALU.mult)
                    for j in range(B2):
                        mm(b0 + j, t_tile[:, j, :])

            pending = []
            for s0 in range(0, len(groups), PG):
                ss = list(range(s0, min(s0 + PG, len(groups))))
                zms = [zm_phase(g) for g in ss]
                es = []
                for g, zm in zip(ss, zms):
                    e = ep.tile([P, B2, FREE], fp16, tag="e")
                    nc.scalar.activation(e[:], zm[:], AF.Derivative_Erf)
                    es.append(e)
                ums = []
                for g, zm in zip(ss, zms):
                    k, _ = groups[g]
                    aa_b = aa_sb[k][:].unsqueeze(1).broadcast_to([P, B2, FREE])
                    um = ump.tile([P, B2, FREE], fp16, tag="um")
                    nc.vector.tensor_tensor(um[:], zm[:], aa_b, op=ALU.mult)
                    ums.append(um)
                # previous superstep's DVE tail work goes here: by now ACT
                # has drained those groups while DVE was busy with zm above
                for args in pending:
                    tail(*args)
                pending = []
                for g, e, um in zip(ss, es, ums):
                    t_tile = tp.tile([P, B2, FREE], fp16, tag="t")
                    nc.scalar.activation(t_tile[:], um[:], AF.Erf)
                    if not TWO_STREAM:
                        nc.scalar.activation(
                            t_tile[:], t_tile[:], AF.Identity, bias=1.0
                        )
                    pending.append((g, e, t_tile))
            for args in pending:
                tail(*args)

            nc.vector.tensor_reduce(
                out_sb[:], psum_t[:], axis=mybir.AxisListType.X, op=ALU.add
            )
            nc.sync.dma_start(out_d[:], out_sb[:])

    nc.compile()
    return nc


def _prep_inputs(x, mx_train, scale, sigma, alpha, w, mx_start):
    c = (np.abs(scale)[:, :, None] * mx_start[None, None, :]
         + mx_train[:, :, None]).astype(np.float32)
    rinv = (1.0 / (np.abs(sigma) + 1e-8)).astype(np.float32)
    cre = (c * rinv).astype(np.float32)
    aa = (alpha * INV_SQRT2).astype(np.float16)
    wt = (w * SQRT_PI_2).astype(np.float16)
    xT = np.ascontiguousarray(x.T.reshape(KC, P, B)).astype(np.float32)
    oh = np.ascontiguousarray(
        np.broadcast_to(np.eye(B, dtype=np.float16), (P, B, B)))

    in_maps = []
    for d in range(NCORES):
        sl = slice(d * OS, (d + 1) * OS)
        in_maps.append({
            "cre": np.ascontiguousarray(cre[:, sl].reshape(KC, P, FREE)),
            "re": np.ascontiguousarray(rinv[:, sl].reshape(KC, P, FREE)),
            "aa": np.ascontiguousarray(aa[:, sl].reshape(KC, P, FREE)),
            "wt": np.ascontiguousarray(wt[:, sl].reshape(KC, P, FREE)),
            "x": xT,
            "oh": oh,
        })
    return in_maps


def kernel(x, mx_train, scale, sigma, alpha, w, mx_start, _trace=False):
    global LAST_RESULTS
    from concourse.bass_utils import run_bass_kernel_spmd

    if "nc" not in _CACHE:
        _CACHE["nc"] = _build_nc()
    nc = _CACHE["nc"]
    in_maps = _prep_inputs(
        np.asarray(x, np.float32), np.asarray(mx_train, np.float32),
        np.asarray(scale, np.float32), np.asarray(sigma, np.float32),
        np.asarray(alpha, np.float32), np.asarray(w, np.float32),
        np.asarray(mx_start, np.float32),
    )
    res = run_bass_kernel_spmd(nc, in_maps, core_ids=list(range(NCORES)),
                               trace=_trace)
    LAST_RESULTS = res
    return np.concatenate([r["out"] for r in res.results], axis=1)


# revision 8
# speedup vs baseline: 1.5731x; 1.0012x over previous
"""Trainium2 Bass kernel for nn_KATLayer (KAT basis-function layer).

out[b,o] = sum_{i,n} exp(-z^2) * (1 + erf(alpha*z/sqrt(2))) * w[i,o,n]
  z = (x[b,i] - c[i,o,n]) / (|sigma|+1e-8),  c = |scale|*mx_start + mx_train

Sharding: output dim O split across 8 cores (OS=64 per core). Per core the
16.7M basis evaluations are tiled as partitions=i (KC=4 chunks of 128),
free=(o_local,n)=1024, one logical tile per (k,b); tiles run in groups of
B2=4 consecutive b sharing one k so per-k constants broadcast across the
group dim with stride-0 APs and fp16 DVE ops run 4096-wide at 2x rate.

Per group (host precomputes re32=rinv fp32, cre32=c*rinv fp32,
aa=alpha/sqrt2 fp16, wt=w*sqrt(pi)/2 fp16):
  zm = re32*x_b - cre32    (4x STT fp32->fp16; zm = +z, high precision)
  e  = DerivErf(zm)        (ACT)          um = zm*aa  (TT fp16 2x)
  t  = Erf(um)             (ACT)
  t1 = t+1   (TS fp16 imm,imm 4x, in-place on t)
  q  = e*t1  (TT 2x, out in t-tile)   p = q*wt (TT 2x in-place / GPSIMD)
  psum[b,(o,n)] += onehot_b.T @ p     (PE reduces i)
  out = reduce_n(psum)                (DVE, once)

Activations are phase-batched PG groups at a time (all DerivErf, then all
Erf) to amortize the ~1.3us ACT table switch; the DVE tail work (t1/q/p)
of superstep s is emitted after the zm-phase of superstep s+1 so the
in-order DVE queue never stalls waiting on ACT. p runs on GPSIMD (plain
fp16 TT) to off-load the DVE, which is the bottleneck engine.
"""
import sys

sys.path.insert(0, "/opt/trn_rl_repo")
import math

import numpy as np

B, I, O, N = 32, 512, 512, 16
NCORES = 8
OS = O // NCORES          # 64 output dims per core
KC = I // 128             # 4 i-chunks
P = 128
FREE = OS * N             # 1024
B2 = 4                    # b-tiles fused per instruction group
PG = 3                    # groups per activation phase batch
INV_SQRT2 = 0.7071067811865476
SQRT_PI_2 = math.sqrt(math.pi) / 2.0

TWO_STREAM = True         # accumulate S(e*w)*t + S(e*w) via 2x matmuls

_CACHE = {}
LAST_RESULTS = None


def _build_nc():
    import concourse.bacc as bacc
    import concourse.mybir as mybir
    from concourse import tile

    fp32 = mybir.dt.float32
    fp16 = mybir.dt.float16
    AF = mybir.ActivationFunctionType
    ALU = mybir.AluOpType

    nc = bacc.Bacc(
        "TRN2", target_bir_lowering=False, debug=False, num_devices=NCORES
    )
    cre_d = nc.dram_tensor("cre", [KC, P, FREE], fp32, kind="ExternalInput")
    re_d = nc.dram_tensor("re", [KC, P, FREE], fp32, kind="ExternalInput")
    aa_d = nc.dram_tensor("aa", [KC, P, FREE], fp16, kind="ExternalInput")
    wt_d = nc.dram_tensor("wt", [KC, P, FREE], fp16, kind="ExternalInput")
    x_d = nc.dram_tensor("x", [KC, P, B], fp32, kind="ExternalInput")
    oh_d = nc.dram_tensor("oh", [P, B, B], fp16, kind="ExternalInput")
    out_d = nc.dram_tensor("out", [B, OS], fp32, kind="ExternalOutput")

    groups = [(k, B2 * bg) for k in range(KC) for bg in range(B // B2)]
    n_tiles = KC * B

    with tile.TileContext(nc) as tc:
        with (
            tc.tile_pool(name="const", bufs=1) as cp,
            tc.tile_pool(name="zmp", bufs=PG + 1) as zmp,
            tc.tile_pool(name="ump", bufs=PG) as ump,
            tc.tile_pool(name="ep", bufs=2 * PG) as ep,
            tc.tile_pool(name="tp", bufs=2 * PG) as tp,
            tc.tile_pool(name="psum", bufs=1, space="PSUM") as psp,
            tc.tile_pool(name="outp", bufs=1) as op_,
        ):
            # x + oh first (first STT needs x), then per-k in consumption
            # order so the first superstep's inputs land earliest
            x_sb = cp.tile([P, KC * B], fp32, tag="x")
            for k in range(KC):
                nc.sync.dma_start(x_sb[:, k * B : (k + 1) * B], x_d[k])
            oh_sb = cp.tile([P, B, B], fp16, tag="oh")
            nc.sync.dma_start(oh_sb[:], oh_d[:])
            cre_sb, re_sb, aa_sb, wt_sb = [], [], [], []
            for k in range(KC):
                for lst, dram, nm, dt_ in (
                    (cre_sb, cre_d, "cre", fp32),
                    (re_sb, re_d, "re", fp32),
                    (aa_sb, aa_d, "aa", fp16),
                    (wt_sb, wt_d, "wt", fp16),
                ):
                    t = cp.tile([P, FREE], dt_, tag=f"{nm}{k}")
                    nc.sync.dma_start(t[:], dram[k])
                    lst.append(t)

            psum_t = psp.tile([B, OS, N], fp32)
            out_sb = op_.tile([B, OS], fp32)

            state = {"n_mm": 0}

            def zm_phase(g):
                k, b0 = groups[g]
                zm = zmp.tile([P, B2, FREE], fp16, tag="zm")
                for j in range(B2):
                    xcol = x_sb[:, k * B + b0 + j : k * B + b0 + j + 1]
                    nc.vector.scalar_tensor_tensor(
                        zm[:, j, :], re_sb[k][:], xcol, cre_sb[k][:],
                        op0=ALU.mult, op1=ALU.subtract,
                    )
                return zm

            n_mm_total = (4 if TWO_STREAM else 2) * n_tiles

            def mm(b, mov):
                for h in range(2):
                    nc.tensor.matmul(
                        psum_t[:, 32 * h : 32 * (h + 1), :],
                        oh_sb[:, b, :],
                        mov[:, 512 * h : 512 * (h + 1)],
                        start=(state["n_mm"] < 2),
                        stop=(state["n_mm"] >= n_mm_total - 2),
                    )
                    state["n_mm"] += 1

            def tail(g, e, t_tile):
                k, b0 = groups[g]
                wt_b = wt_sb[k][:].unsqueeze(1).broadcast_to([P, B2, FREE])
                if TWO_STREAM:
                    # ew = e*wt (in e-tile); s2 = ew*t (in t-tile)
                    nc.vector.tensor_tensor(e[:], e[:], wt_b, op=ALU.mult)
                    nc.vector.tensor_tensor(
                        t_tile[:], e[:], t_tile[:], op=ALU.mult)
                    for j in range(B2):
                        mm(b0 + j, t_tile[:, j, :])
                        mm(b0 + j, e[:, j, :])
                else:
                    nc.vector.tensor_tensor(
                        t_tile[:], e[:], t_tile[:], op=ALU.mult)
                    nc.vector.tensor_tensor(
                        t_tile[:], t_tile[:], wt_b, op=ALU.mult)
                    for j in range(B2):
                        mm(b0 + j, t_tile[:, j, :])

            pending = []
            for s0 in range(0, len(groups), PG):
                ss = list(range(s0, min(s0 + PG, len(groups))))
                zms = [zm_phase(g) for g in ss]
                es = []
                for g, zm in zip(ss, zms):
                    e = ep.tile([P, B2, FREE], fp16, tag="e")
                    nc.scalar.activation(e[:], zm[:], AF.Derivative_Erf)
                    es.append(e)
                ums = []
                for g, zm in zip(ss, zms):
                    k, _ = groups[g]
                    aa_b = aa_sb[k][:].unsqueeze(1).broadcast_to([P, B2, FREE])
                    um = ump.tile([P, B2, FREE], fp16, tag="um")
                    nc.vector.tensor_tensor(um[:], zm[:], aa_b, op=ALU.mult)
                    ums.append(um)
                # previous superstep's DVE tail work goes here: by now ACT
                # has drained those groups while DVE was busy with zm above
                for args in pending:
                    tail(*args)
                pending = []
                for g, e, um in zip(ss, es, ums):
                    t_tile = tp.tile([P, B2, FREE], fp16, tag="t")
                    nc.scalar.activation(t_tile[:], um[:], AF.Erf)
                    if not TWO_STREAM:
                        nc.scalar.activation(
                            t_tile[:], t_tile[:], AF.Identity, bias=1.0
                        )
                    pending.append((g, e, t_tile))
            for args in pending:
                tail(*args)

            nc.vector.tensor_reduce(
                out_sb[:], psum_t[:], axis=mybir.AxisListType.X, op=ALU.add
            )
            nc.sync.dma_start(out_d[:], out_sb[:])

    nc.compile()
    return nc


def _prep_inputs(x, mx_train, scale, sigma, alpha, w, mx_start):
    c = (np.abs(scale)[:, :, None] * mx_start[None, None, :]
         + mx_train[:, :, None]).astype(np.float32)
    rinv = (1.0 / (np.abs(sigma) + 1e-8)).astype(np.float32)
    cre = (c * rinv).astype(np.float32)
    aa = (alpha * INV_SQRT2).astype(np.float16)
    wt = (w * SQRT_PI_2).astype(np.float16)
    xT = np.ascontiguousarray(x.T.reshape(KC, P, B)).astype(np.float32)
    oh = np.ascontiguousarray(
        np.broadcast_to(np.eye(B, dtype=np.float16), (P, B, B)))

    in_maps = []
    for d in range(NCORES):
        sl = slice(d * OS, (d + 1) * OS)
        in_maps.append({
            "cre": np.ascontiguousarray(cre[:, sl].reshape(KC, P, FREE)),
            "re": np.ascontiguousarray(rinv[:, sl].reshape(KC, P, FREE)),
            "aa": np.ascontiguousarray(aa[:, sl].reshape(KC, P, FREE)),
            "wt": np.ascontiguousarray(wt[:, sl].reshape(KC, P, FREE)),
            "x": xT,
            "oh": oh,
        })
    return in_maps


def kernel(x, mx_train, scale, sigma, alpha, w, mx_start, _trace=False):
    global LAST_RESULTS
    from concourse.bass_utils import run_bass_kernel_spmd

    if "nc" not in _CACHE:
        _CACHE["nc"] = _build_nc()
    nc = _CACHE["nc"]
    in_maps = _prep_inputs(
        np.asarray(x, np.float32), np.asarray(mx_train, np.float32),
        np.asarray(scale, np.float32), np.asarray(sigma, np.float32),
        np.asarray(alpha, np.float32), np.asarray(w, np.float32),
        np.asarray(mx_start, np.float32),
    )
    res = run_bass_kernel_spmd(nc, in_maps, core_ids=list(range(NCORES)),
                               trace=_trace)
    LAST_RESULTS = res
    return np.concatenate([r["out"] for r in res.results], axis=1)
